# revision 7
# baseline (speedup 1.0000x reference)
"""Multi-head attention (B=4, N=1024, C=1024, H=16) on 8 TRN2 NeuronCores.

Sharding: core c handles batch b = c//2 and query-row half g = c%2.
Data parallel over B; within a batch pair, tensor parallel over heads for
the K/V projections (each core computes K,V for its 8 heads over all 1024
keys, then 2-core AllGathers exchange them), while Q is computed for the
core's own 512 query rows over all 16 heads.  Each core then runs full
attention + output projection for its 512 rows — output rows are disjoint,
so no all-reduce is needed after the projection.

Compute is bf16 on the TensorEngine with fp32 PSUM accumulation; softmax
is computed without max-subtraction (logits are bounded ~2.5 for this
problem) as exp(S)@[V*e, e] followed by a per-partition divide, where
e = exp(-5*(1-mask)) folds the additive mask penalty in exactly.
"""

import numpy as np
import ml_dtypes

import concourse.bass as bass
import concourse.mybir as mybir
import concourse.tile as tile
from concourse import bacc

N_CORES = 8
B, N, C = 4, 1024, 1024
H = 16
D = C // H  # 64
NQ = N // 2  # query rows per core: 512
P = 128
KC = C // P  # 8 contraction chunks
SCALE = D ** -0.5
PAIRS = [[0, 1], [2, 3], [4, 5], [6, 7]]

F32 = mybir.dt.float32
BF16 = mybir.dt.bfloat16
AF = mybir.ActivationFunctionType


def build_nc():
    nc = bacc.Bacc(None, num_devices=N_CORES)

    xT = nc.declare_dram_parameter("xT", [C, N], BF16, isOutput=False)
    xqT = nc.declare_dram_parameter("xqT", [C, NQ], BF16, isOutput=False)
    W_q = nc.declare_dram_parameter("W_q", [C, C], BF16, isOutput=False)
    W_k = nc.declare_dram_parameter("W_k", [C, C // 2], BF16, isOutput=False)
    W_v = nc.declare_dram_parameter("W_v", [C, C // 2], BF16, isOutput=False)
    W_p = nc.declare_dram_parameter("W_p", [C, C], BF16, isOutput=False)
    e_in = nc.declare_dram_parameter("e", [N], F32, isOutput=False)
    b_in = nc.declare_dram_parameter("b", [C], F32, isOutput=False)
    out_ext = nc.declare_dram_parameter("out", [NQ, C], F32, isOutput=True)

    # Internal DRAM for the pairwise K/V exchanges (bf16).
    ccK_in = nc.dram_tensor("ccK_in", [C // 2, N], BF16)   # K^T own
    ccK_out = nc.dram_tensor("ccK_out", [2, C // 2, N], BF16)
    ccV_in = nc.dram_tensor("ccV_in", [N, C // 2], BF16)   # V own (mask-scaled)
    ccV_out = nc.dram_tensor("ccV_out", [2, N, C // 2], BF16)

    with tile.TileContext(nc) as tc:
        with (
            tc.tile_pool(name="weights", bufs=1) as wpool,
            tc.tile_pool(name="acts", bufs=1) as apool,
            tc.tile_pool(name="work", bufs=2) as work,
            tc.tile_pool(name="ps_mm", bufs=2, space="PSUM") as ps_mm,
            tc.tile_pool(name="ps_st", bufs=2, space="PSUM") as ps_st,
            tc.tile_pool(name="ps_pv", bufs=2, space="PSUM") as ps_pv,
        ):
            # ---- static loads (chunked so matmuls can start early) -------
            Wk_s = wpool.tile([P, KC, C // 2], BF16)
            xT_s = apool.tile([P, KC, N], BF16)
            Wv_s = wpool.tile([P, KC, C // 2], BF16)
            for kc in range(KC):
                nc.sync.dma_start(Wk_s[:, kc, :], W_k[kc * P:(kc + 1) * P, :])
                nc.sync.dma_start(xT_s[:, kc, :], xT[kc * P:(kc + 1) * P, :])
            for kc in range(KC):
                nc.sync.dma_start(Wv_s[:, kc, :], W_v[kc * P:(kc + 1) * P, :])
            e_s = apool.tile([P, KC], F32)
            nc.sync.dma_start(e_s[:], e_in.rearrange("(o p) -> p o", p=P))

            # ---- K^T own: [feat 512, keys 1024] --------------------------
            kt_stage = apool.tile([P, 4, N], BF16)
            for fc in range(4):
                for nh in range(2):  # keys halves of 512
                    ps = ps_mm.tile([P, 512], F32, tag="mm")
                    for kc in range(KC):
                        nc.tensor.matmul(
                            ps[:],
                            Wk_s[:, kc, fc * P:(fc + 1) * P],
                            xT_s[:, kc, nh * 512:(nh + 1) * 512],
                            start=(kc == 0),
                            stop=(kc == KC - 1),
                        )
                    nc.vector.tensor_copy(
                        kt_stage[:, fc, nh * 512:(nh + 1) * 512], ps[:]
                    )
                nc.sync.dma_start(
                    ccK_in[fc * P:(fc + 1) * P, :], kt_stage[:, fc, :]
                )

            nc.gpsimd.collective_compute(
                "AllGather",
                mybir.AluOpType.bypass,
                replica_groups=PAIRS,
                ins=[ccK_in[:].opt()],
                outs=[ccK_out[:].opt()],
            )

            # ---- V own (mask-scaled): [keys 1024, feat 512] --------------
            v_stage = apool.tile([P, KC, 512], BF16)
            for mc in range(KC):
                ps = ps_mm.tile([P, 512], F32, tag="mm")
                for kc in range(KC):
                    nc.tensor.matmul(
                        ps[:],
                        xT_s[:, kc, mc * P:(mc + 1) * P],
                        Wv_s[:, kc, :],
                        start=(kc == 0),
                        stop=(kc == KC - 1),
                    )
                nc.vector.tensor_scalar_mul(
                    v_stage[:, mc, :], ps[:], e_s[:, mc:mc + 1]
                )
                nc.sync.dma_start(
                    ccV_in[mc * P:(mc + 1) * P, :], v_stage[:, mc, :]
                )

            nc.gpsimd.collective_compute(
                "AllGather",
                mybir.AluOpType.bypass,
                replica_groups=PAIRS,
                ins=[ccV_in[:].opt()],
                outs=[ccV_out[:].opt()],
            )

            # ---- Q^T (overlaps the collectives): [feat 1024, q 512] ------
            Wq_s = wpool.tile([P, KC, C], BF16)
            xqT_s = apool.tile([P, KC, NQ], BF16)
            for kc in range(KC):
                nc.sync.dma_start(Wq_s[:, kc, :], W_q[kc * P:(kc + 1) * P, :])
                nc.sync.dma_start(xqT_s[:, kc, :], xqT[kc * P:(kc + 1) * P, :])
            QT_s = apool.tile([P, KC, NQ], BF16)
            for fc in range(KC):
                ps = ps_mm.tile([P, 512], F32, tag="mm")
                for kc in range(KC):
                    nc.tensor.matmul(
                        ps[:],
                        Wq_s[:, kc, fc * P:(fc + 1) * P],
                        xqT_s[:, kc, :],
                        start=(kc == 0),
                        stop=(kc == KC - 1),
                    )
                nc.vector.tensor_copy(QT_s[:, fc, :], ps[:])

            # late weight loads (overlap collective / attention)
            Wp_s = wpool.tile([P, KC, C], BF16)
            nc.sync.dma_start(Wp_s[:], W_p.rearrange("(ko p) n -> p ko n", p=P))
            bias_s = apool.tile([P, C], F32)
            nc.sync.dma_start(bias_s[:], b_in[None, :].to_broadcast((P, C)))

            # ---- gather K^T/V from both pair members ---------------------
            KT_s = apool.tile([P, KC, N], BF16)
            V_s = apool.tile([P, KC, H, D + 1], BF16)
            for r in range(2):
                for fc in range(4):
                    nc.sync.dma_start(
                        KT_s[:, r * 4 + fc, :],
                        ccK_out[r, fc * P:(fc + 1) * P, :],
                    )
                for mc in range(KC):
                    nc.sync.dma_start(
                        V_s[:, mc, r * 8:(r + 1) * 8, 0:D],
                        ccV_out[r, mc * P:(mc + 1) * P, :].rearrange(
                            "p (h d) -> p h d", d=D
                        ),
                    )
            for mc in range(KC):
                nc.vector.tensor_copy(
                    V_s[:, mc, :, D:D + 1],
                    e_s[:, mc:mc + 1, None].to_broadcast((P, H, 1)),
                )

            # ---- attention: head pairs -----------------------------------
            PT_s = apool.tile([P, KC, NQ], BF16)  # attnO^T for the projection
            for hp in range(KC):
                expT = [
                    work.tile([P, KC, NQ], BF16, tag=f"exp{h01}",
                              name=f"expT{h01}")
                    for h01 in range(2)
                ]
                # S^T tiles + exp, two key-chunks per PSUM tile
                for kcp in range(4):
                    for h01 in range(2):
                        lo, hi = h01 * 64, h01 * 64 + 64
                        ps = ps_st.tile([P, 1024], F32, tag="st")
                        for j in range(2):
                            kc = 2 * kcp + j
                            nc.tensor.matmul(
                                ps[:, j * 512:(j + 1) * 512],
                                KT_s[lo:hi, hp, kc * P:(kc + 1) * P],
                                QT_s[lo:hi, hp, :],
                                start=True,
                                stop=True,
                            )
                        nc.scalar.activation(
                            expT[h01][:, 2 * kcp:2 * kcp + 2, :],
                            ps[:].rearrange("p (a b) -> p a b", a=2),
                            AF.Exp,
                            scale=SCALE,
                        )
                # PV with V stationary: out O^T [65, q]; row 64 = denominator
                for h01 in range(2):
                    h = 2 * hp + h01
                    pv = ps_pv.tile([P, NQ], F32, tag="pv")
                    for kc in range(KC):
                        nc.tensor.matmul(
                            pv[0:D + 1, :],
                            V_s[:, kc, h, :],
                            expT[h01][:, kc, :],
                            start=(kc == 0),
                            stop=(kc == KC - 1),
                        )
                    recip = work.tile([1, NQ], F32, tag="recip")
                    nc.vector.reciprocal(recip[0:1, :], pv[D:D + 1, :])
                    bcast = work.tile([D, NQ], F32, tag="bcast")
                    nc.gpsimd.partition_broadcast(bcast[:], recip[0:1, :])
                    nc.vector.tensor_mul(
                        PT_s[h01 * D:(h01 + 1) * D, hp, :],
                        pv[0:D, :],
                        bcast[:],
                    )

            # ---- output projection + bias --------------------------------
            for qs in range(4):
                for nn in range(2):
                    ps = ps_mm.tile([P, 512], F32, tag="mm")
                    for fc in range(KC):
                        nc.tensor.matmul(
                            ps[:],
                            PT_s[:, fc, qs * P:(qs + 1) * P],
                            Wp_s[:, fc, nn * 512:(nn + 1) * 512],
                            start=(fc == 0),
                            stop=(fc == KC - 1),
                        )
                    o_sb = work.tile([P, 512], F32, tag="osb")
                    nc.vector.tensor_add(
                        o_sb[:], ps[:], bias_s[:, nn * 512:(nn + 1) * 512]
                    )
                    nc.sync.dma_start(
                        out_ext[qs * P:(qs + 1) * P, nn * 512:(nn + 1) * 512],
                        o_sb[:],
                    )

    nc.finalize()
    return nc


def make_in_maps(x, mask, W_qkv, W_proj, b_proj):
    bf = ml_dtypes.bfloat16
    x = np.asarray(x, np.float32)
    mask = np.asarray(mask, np.float32)
    W_qkv = np.asarray(W_qkv, np.float32)
    W_proj = np.asarray(W_proj, np.float32)
    b_proj = np.asarray(b_proj, np.float32)

    W_q = np.ascontiguousarray(W_qkv[:, 0:C]).astype(bf)
    W_p = np.ascontiguousarray(W_proj).astype(bf)
    e_all = np.exp(-5.0 * (1.0 - mask)).astype(np.float32)  # [B, N]

    in_maps = []
    for c in range(N_CORES):
        b, g = divmod(c, 2)
        xT = np.ascontiguousarray(x[b].T).astype(bf)
        xqT = np.ascontiguousarray(x[b, g * NQ:(g + 1) * NQ, :].T).astype(bf)
        W_k = np.ascontiguousarray(
            W_qkv[:, C + g * (C // 2):C + (g + 1) * (C // 2)]
        ).astype(bf)
        W_v = np.ascontiguousarray(
            W_qkv[:, 2 * C + g * (C // 2):2 * C + (g + 1) * (C // 2)]
        ).astype(bf)
        in_maps.append({
            "xT": xT, "xqT": xqT, "W_q": W_q, "W_k": W_k, "W_v": W_v,
            "W_p": W_p, "e": np.ascontiguousarray(e_all[b]),
            "b": b_proj,
        })
    return in_maps


def assemble_output(results):
    out = np.zeros((B, N, C), np.float32)
    for c in range(N_CORES):
        b, g = divmod(c, 2)
        out[b, g * NQ:(g + 1) * NQ, :] = results[c]["out"]
    return out


def kernel(x, mask, W_qkv, W_proj, b_proj):
    from concourse.bass_utils import run_bass_kernel_spmd

    nc = build_nc()
    in_maps = make_in_maps(x, mask, W_qkv, W_proj, b_proj)
    res = run_bass_kernel_spmd(nc, in_maps, core_ids=list(range(N_CORES)))
    return assemble_output(res.results)
